# revision 9
# baseline (speedup 1.0000x reference)
"""GCN (2-layer, PyG GCNConv semantics) on 8 Trainium2 NeuronCores.

v2 fused single-NEFF design:
  - Nodes partitioned across 8 cores (6250 each), T=49 dst tiles of 128.
  - norm = dinv[src]*dinv[dst] factored: dinv[src] folded into the gather
    sources (xn = x*dinv on host; h1n = relu(h1)*dinv on device), dinv[dst]
    applied per-partition on each final tile; S matrices are 0/1 indicators.
  - Per tile: psum[din,dst] += g_blk^T @ S_blk over the tile's edge blocks,
    then out[dst,dout] = (psum^T @ W)*dinv + b (+relu in layer 1).
  - Layer 1 rows PRE-GATHERED ON HOST into xg, streamed with big DMAs
    alternating between the sync and scalar HWDGE queues.
  - h1n exchanged via FOUR chunked Shared-output AllGathers (pieces of the
    node range), each launched as soon as its L1 tiles finish -> overlapped
    with the L1 tail.
  - Layer 2 re-gathers the edge list from the 4 piece tables with
    dma_gather; blocks laid out piece-major so each (chunk,piece) is one
    large contiguous gather call; calls issued in a wavefront order so the
    Pool engine never stalls on a not-yet-landed AllGather.
  - S one-hot builds alternate between DVE and Pool engines.
"""
import os
import sys
import numpy as np

try:
    import concourse.bass as bass
except ImportError:
    sys.path.insert(0, "/opt/trn_rl_repo")
    import concourse.bass as bass
import concourse.bacc as bacc
import concourse.mybir as mybir
from concourse import tile
from concourse.bass_utils import run_bass_kernel_spmd

N_NODES = 50000
N_EDGES = 800000
D = 128
N_CORES = 8
TILE_N = 128
NP = 4          # number of h1-exchange pieces / tables

DT = mybir.dt.float16
NPDT = np.float16
F32 = mybir.dt.float32

last_exec_time_ns = None


def _ceil_div(a, b):
    return -(-a // b)


class Plan:
    pass


def _preprocess(edge_index: np.ndarray, n_nodes=N_NODES):
    p = Plan()
    npc = n_nodes // N_CORES
    T = _ceil_div(npc, TILE_N)

    # piece boundaries in tiles
    base_tiles = T // NP
    extra = T - base_tiles * NP
    piece_tiles = [base_tiles + (1 if i < extra else 0) for i in range(NP)]
    tile_start = np.concatenate([[0], np.cumsum(piece_tiles)]).astype(np.int64)
    piece_row_start = tile_start * TILE_N
    piece_rows = [
        int(min((tile_start[i + 1]) * TILE_N, npc) - piece_row_start[i])
        for i in range(NP)
    ]
    assert all(N_CORES * r < 32768 for r in piece_rows)

    src = edge_index[0].astype(np.int64)
    dst = edge_index[1].astype(np.int64)
    loops = np.arange(n_nodes, dtype=np.int64)
    src_all = np.concatenate([src, loops])
    dst_all = np.concatenate([dst, loops])
    E = len(src_all)

    deg = np.bincount(dst_all, minlength=n_nodes).astype(np.float32)
    dinv = (1.0 / np.sqrt(deg)).astype(np.float32)

    core = dst_all // npc           # owning (dst) core
    tloc = (dst_all % npc) // TILE_N
    dloc = (dst_all % npc) % TILE_N
    cs = src_all // npc             # src core
    js = src_all % npc              # src row within core
    # piece of the src row
    piece = np.searchsorted(piece_row_start[1:NP], js, side="right")
    rloc = js - piece_row_start[piece]
    gidx = (cs * np.array(piece_rows)[piece] + rloc).astype(np.int64)

    # ---------------- layer-1 grouping: (core, tile) only -----------------
    key1 = core * T + tloc
    order1 = np.argsort(key1, kind="stable")
    cnt1 = np.bincount(key1, minlength=N_CORES * T).reshape(N_CORES, T)
    B1 = _ceil_div(cnt1, TILE_N).max(axis=0)            # [T]
    block_of1 = np.concatenate([[0], np.cumsum(B1)]).astype(np.int64)
    NB1 = int(block_of1[-1])

    g1start = np.concatenate([[0], np.cumsum(cnt1.reshape(-1))[:-1]])
    rank1 = np.arange(E) - g1start[key1[order1]]
    slots1 = block_of1[tloc[order1]] * TILE_N + rank1
    node_flat = np.zeros((N_CORES, NB1 * TILE_N), dtype=np.int64)
    dloc1_flat = np.full((N_CORES, NB1 * TILE_N), -1.0, dtype=np.float32)
    node_flat[core[order1], slots1] = src_all[order1]
    dloc1_flat[core[order1], slots1] = dloc[order1]

    p.node_flat = node_flat
    p.dloc1 = np.ascontiguousarray(
        dloc1_flat.reshape(N_CORES, NB1, TILE_N).transpose(0, 2, 1))

    # ------------- layer-2 grouping: piece-major (piece, tile) ------------
    key2 = (piece * T + tloc) * N_CORES + core          # piece-major sections
    order2 = np.argsort(key2, kind="stable")
    cnt2 = np.bincount(key2, minlength=NP * T * N_CORES)
    cnt2_pt = cnt2.reshape(NP, T, N_CORES)
    Bpt = _ceil_div(cnt2_pt, TILE_N).max(axis=2)        # [NP, T]
    # block offset for (p, t), piece-major
    flat_B = Bpt.reshape(-1)
    block_of2 = np.concatenate([[0], np.cumsum(flat_B)]).reshape(-1)
    NB2 = int(block_of2[-1])
    block_of2_pt = block_of2[:-1].reshape(NP, T)
    sec0 = np.array([block_of2_pt[q, 0] for q in range(NP)] + [NB2])

    g2start = np.concatenate([[0], np.cumsum(cnt2)[:-1]])
    rank2 = np.arange(E) - g2start[key2[order2]]
    slots2 = block_of2_pt[piece[order2], tloc[order2]] * TILE_N + rank2
    idx_flat = np.zeros((N_CORES, NB2 * TILE_N), dtype=np.int64)
    dloc2_flat = np.full((N_CORES, NB2 * TILE_N), -1.0, dtype=np.float32)
    idx_flat[core[order2], slots2] = gidx[order2]
    dloc2_flat[core[order2], slots2] = dloc[order2]

    cols = NB2 * TILE_N // 16
    base = idx_flat.reshape(N_CORES, cols, 16).transpose(0, 2, 1)
    p.idx_wrapped = np.ascontiguousarray(
        np.tile(base, (1, 8, 1)).astype(np.int16))
    p.dloc2 = np.ascontiguousarray(
        dloc2_flat.reshape(N_CORES, NB2, TILE_N).transpose(0, 2, 1))

    dv = np.ones((N_CORES, T * TILE_N), np.float32)
    dv[:, :npc] = dinv.reshape(N_CORES, npc)
    p.dinv_cols = np.ascontiguousarray(
        dv.reshape(N_CORES, T, TILE_N).transpose(0, 2, 1))

    p.n_nodes, p.npc, p.T = n_nodes, npc, T
    p.piece_tiles, p.tile_start = piece_tiles, tile_start
    p.piece_rows, p.piece_row_start = piece_rows, piece_row_start
    p.NB1, p.B1, p.block_of1 = NB1, B1, block_of1
    p.NB2, p.Bpt, p.block_of2_pt, p.sec0 = NB2, Bpt, block_of2_pt, sec0
    p.dinv = dinv
    return p


def _sig(p: Plan):
    return (p.NB1, p.NB2, tuple(p.B1.tolist()),
            tuple(p.Bpt.reshape(-1).tolist()))


def _build_nc(p: Plan, chunk_tiles=5):
    USE_ACT_DMA = bool(int(os.environ.get('GCN_ACT_DMA', '0')))
    USE_STT = bool(int(os.environ.get('GCN_STT', '0')))
    T, NB1, NB2 = p.T, p.NB1, p.NB2
    npc = p.npc
    B1, block_of1 = p.B1, p.block_of1
    Bpt, block_of2_pt = p.Bpt, p.block_of2_pt
    piece_rows = p.piece_rows
    tile_start = p.tile_start

    nc = bacc.Bacc("TRN2", target_bir_lowering=False, debug=False,
                   num_devices=N_CORES, num_swdge_queues=4)

    xg_dram = nc.dram_tensor("xg", [128, NB1, D], DT, kind="ExternalInput").ap()
    w1_dram = nc.dram_tensor("w1", [D, D], DT, kind="ExternalInput").ap()
    w2_dram = nc.dram_tensor("w2", [D, D], DT, kind="ExternalInput").ap()
    b1_dram = nc.dram_tensor("b1", [128, D], F32, kind="ExternalInput").ap()
    b2_dram = nc.dram_tensor("b2", [128, D], F32, kind="ExternalInput").ap()
    iota_dram = nc.dram_tensor("iota", [128, 128], F32, kind="ExternalInput").ap()
    idx_dram = nc.dram_tensor("idx", [128, NB2 * 8], mybir.dt.int16,
                              kind="ExternalInput").ap()
    dloc1_dram = nc.dram_tensor("dloc1", [128, NB1], F32, kind="ExternalInput").ap()
    dloc2_dram = nc.dram_tensor("dloc2", [128, NB2], F32, kind="ExternalInput").ap()
    dinv_dram = nc.dram_tensor("dinv", [128, T], F32, kind="ExternalInput").ap()
    out_dram = nc.dram_tensor("out", [npc, D], F32, kind="ExternalOutput").ap()

    # tile chunks (shared by both layers)
    chunks = []
    for c0 in range(0, T, chunk_tiles):
        chunks.append(list(range(c0, min(c0 + chunk_tiles, T))))
    NCH = len(chunks)

    with tile.TileContext(nc) as tc:
        with (
            tc.tile_pool(name="resident", bufs=1) as rpool,
            tc.tile_pool(name="l1g", bufs=2) as l1pool,
            tc.tile_pool(name="gbuf", bufs=12) as gpool,
            tc.tile_pool(name="s", bufs=3) as spool,
            tc.tile_pool(name="agg", bufs=3) as apool,
            tc.tile_pool(name="hout", bufs=4) as hpool,
            tc.tile_pool(name="psum_acc", bufs=4, space="PSUM") as pacc,
            tc.tile_pool(name="psum_mm", bufs=2, space="PSUM") as pmm,
            tc.tile_pool(name="dram", bufs=1, space="DRAM") as dpool,
        ):
            # residents
            dloc1_t = rpool.tile([128, NB1], F32)
            nc.sync.dma_start(dloc1_t[:], dloc1_dram[:])
            dloc2_t = rpool.tile([128, NB2], F32)
            nc.sync.dma_start(dloc2_t[:], dloc2_dram[:])
            iota_t = rpool.tile([128, 128], F32)
            nc.sync.dma_start(iota_t[:], iota_dram[:])
            w1_t = rpool.tile([D, D], DT)
            nc.sync.dma_start(w1_t[:], w1_dram[:])
            w2_t = rpool.tile([D, D], DT)
            nc.sync.dma_start(w2_t[:], w2_dram[:])
            b1_t = rpool.tile([128, D], F32)
            nc.sync.dma_start(b1_t[:], b1_dram[:])
            b2_t = rpool.tile([128, D], F32)
            nc.sync.dma_start(b2_t[:], b2_dram[:])
            dinv_t = rpool.tile([128, T], F32)
            nc.sync.dma_start(dinv_t[:], dinv_dram[:])
            idx_t = rpool.tile([128, NB2 * 8], mybir.dt.int16)
            nc.sync.dma_start(idx_t[:], idx_dram[:])

            h1loc = [dpool.tile([piece_rows[q], D], DT, name=f"h1loc{q}")
                     for q in range(NP)]
            tab_space = os.environ.get("GCN_TAB_SPACE", "Shared")
            tabs = [dpool.tile([N_CORES * piece_rows[q], D], DT,
                               name=f"tab{q}", addr_space=tab_space)
                    for q in range(NP)]

            s_eng = [0]

            def emit_s(s_t, off, nbl, dloc_t, b0):
                """one is_equal batch: s_t[:, off:off+nbl, :] one-hot."""
                if nbl == 0:
                    return
                eng = nc.vector
                s_eng[0] += 1
                eng.tensor_tensor(
                    s_t[:, off:off + nbl, :],
                    iota_t[:].unsqueeze(1).to_broadcast([128, int(nbl), 128]),
                    dloc_t[:, b0:b0 + nbl].unsqueeze(2)
                    .to_broadcast([128, int(nbl), 128]),
                    mybir.AluOpType.is_equal,
                )

            # ---------------- layer 1: stream + aggregate ----------------
            ag_done = 0
            for ci, tl in enumerate(chunks):
                nb0, nb1 = int(block_of1[tl[0]]), int(block_of1[tl[-1] + 1])
                g_t = l1pool.tile([128, nb1 - nb0, D], DT, tag="l1g")
                deng = nc.sync if (ci % 2 == 0 or not USE_ACT_DMA) else nc.scalar
                deng.dma_start(g_t[:], xg_dram[:, nb0:nb1, :])

                for t in tl:
                    rows = min(TILE_N, npc - t * TILE_N)
                    nblk = int(B1[t])
                    bh0 = int(block_of1[t])
                    s_t = spool.tile([128, nblk, 128], DT, tag="s")
                    emit_s(s_t, 0, nblk, dloc1_t, bh0)
                    psum = pacc.tile([128, 128], F32, tag="pa")
                    for j in range(nblk):
                        nc.tensor.matmul(
                            psum[:], lhsT=g_t[:, bh0 + j - nb0, :],
                            rhs=s_t[:, j, :],
                            start=(j == 0), stop=(j == nblk - 1),
                        )
                    aggT = apool.tile([128, 128], DT, tag="agg")
                    nc.scalar.activation(
                        aggT[:], psum[:], mybir.ActivationFunctionType.Identity)
                    psum2 = pmm.tile([128, 128], F32, tag="pm")
                    nc.tensor.matmul(psum2[:], lhsT=aggT[:], rhs=w1_t[:],
                                     start=True, stop=True)
                    # h1n = relu(psum2*dinv + b1) * dinv
                    t2 = hpool.tile([128, 128], F32, tag="t2")
                    if USE_STT:
                        nc.vector.scalar_tensor_tensor(
                            t2[:], psum2[:], dinv_t[:, t:t + 1], b1_t[:],
                            mybir.AluOpType.mult, mybir.AluOpType.add)
                    else:
                        t1 = hpool.tile([128, 128], F32, tag="t1")
                        nc.vector.tensor_scalar(
                            t1[:], psum2[:], dinv_t[:, t:t + 1], None,
                            mybir.AluOpType.mult)
                        nc.vector.tensor_tensor(
                            t2[:], t1[:], b1_t[:], mybir.AluOpType.add)
                    h_t = hpool.tile([128, 128], DT, tag="h")
                    nc.scalar.activation(
                        h_t[:], t2[:], mybir.ActivationFunctionType.Relu,
                        scale=dinv_t[:, t:t + 1])
                    q = int(np.searchsorted(tile_start[1:NP + 1], t,
                                            side="right"))
                    r0 = t * TILE_N - int(p.piece_row_start[q])
                    nc.sync.dma_start(
                        h1loc[q][r0:r0 + rows, :], h_t[0:rows, :])

                # launch AllGather for any piece fully produced
                while ag_done < NP and tile_start[ag_done + 1] - 1 <= tl[-1]:
                    qq = ag_done
                    nc.gpsimd.collective_compute(
                        "AllGather", mybir.AluOpType.bypass,
                        replica_groups=[list(range(N_CORES))],
                        ins=[h1loc[qq].opt()], outs=[tabs[qq].opt()],
                    )
                    ag_done += 1

            # ----------- layer 2 gathers: chunk-major, capped calls --------
            GMAX = int(os.environ.get("GCN_GMAX", "8"))
            gtile_of = {}        # global block idx -> (tile_ap, base_block)
            for ci, tl in enumerate(chunks):
                for q in range(NP):
                    gb0 = int(block_of2_pt[q, tl[0]])
                    gb1 = int(block_of2_pt[q, tl[-1]] + Bpt[q, tl[-1]])
                    for s0 in range(gb0, gb1, GMAX):
                        s1 = min(s0 + GMAX, gb1)
                        ncall = s1 - s0
                        g_t = gpool.tile([128, GMAX, D], DT, tag="g")
                        nc.gpsimd.dma_gather(
                            out_ap=g_t[:, 0:ncall, :],
                            in_ap=tabs[q][:],
                            idxs_ap=idx_t[:, s0 * 8:s1 * 8],
                            num_idxs=ncall * TILE_N,
                            num_idxs_reg=ncall * TILE_N,
                            elem_size=D,
                            queue_num=q % 4,
                        )
                        for gb in range(s0, s1):
                            gtile_of[gb] = (g_t, s0)

            # -------------------- layer 2 compute -------------------------
            for ci, tl in enumerate(chunks):
                for t in tl:
                    rows = min(TILE_N, npc - t * TILE_N)
                    nblk = int(Bpt[:, t].sum())
                    s_t = spool.tile([128, nblk, 128], DT, tag="s")
                    off = 0
                    srcs = []
                    for q in range(NP):
                        nq = int(Bpt[q, t])
                        if nq == 0:
                            continue
                        b0 = int(block_of2_pt[q, t])
                        emit_s(s_t, off, nq, dloc2_t, b0)
                        for j in range(nq):
                            g_t, gb0 = gtile_of[b0 + j]
                            srcs.append((g_t, b0 + j - gb0, off + j))
                        off += nq
                    psum = pacc.tile([128, 128], F32, tag="pa")
                    for i, (g_t, gi, si) in enumerate(srcs):
                        nc.tensor.matmul(
                            psum[:], lhsT=g_t[:, gi, :], rhs=s_t[:, si, :],
                            start=(i == 0), stop=(i == len(srcs) - 1),
                        )
                    aggT = apool.tile([128, 128], DT, tag="agg")
                    nc.scalar.activation(
                        aggT[:], psum[:], mybir.ActivationFunctionType.Identity)
                    psum2 = pmm.tile([128, 128], F32, tag="pm")
                    nc.tensor.matmul(psum2[:], lhsT=aggT[:], rhs=w2_t[:],
                                     start=True, stop=True)
                    o_t = hpool.tile([128, 128], F32, tag="o")
                    if USE_STT:
                        nc.vector.scalar_tensor_tensor(
                            o_t[:], psum2[:], dinv_t[:, t:t + 1], b2_t[:],
                            mybir.AluOpType.mult, mybir.AluOpType.add)
                    else:
                        t1b = hpool.tile([128, 128], F32, tag="t1")
                        nc.vector.tensor_scalar(
                            t1b[:], psum2[:], dinv_t[:, t:t + 1], None,
                            mybir.AluOpType.mult)
                        nc.vector.tensor_tensor(
                            o_t[:], t1b[:], b2_t[:], mybir.AluOpType.add)
                    nc.sync.dma_start(
                        out_dram[t * TILE_N:t * TILE_N + rows, :],
                        o_t[0:rows, :])

    nc.compile()
    return nc


_compiled = None


def _kernel_device(x, edge_index, W1, b1, W2, b2, trace=False, tmpdir=None):
    global _compiled, last_exec_time_ns
    ei = np.asarray(edge_index)
    x = np.asarray(x)
    plan = _preprocess(ei, n_nodes=x.shape[0])
    sig = _sig(plan)
    if _compiled is None or _compiled[0] != sig:
        _compiled = (sig, _build_nc(plan))
    nc = _compiled[1]

    iota = np.broadcast_to(np.arange(128, dtype=np.float32), (128, 128)).copy()
    xn = (np.asarray(x, np.float32) * plan.dinv[:, None]).astype(NPDT)
    w1_16 = np.asarray(W1, np.float32).astype(NPDT)
    w2_16 = np.asarray(W2, np.float32).astype(NPDT)
    b1_r = np.ascontiguousarray(np.broadcast_to(
        np.asarray(b1, np.float32), (128, D)))
    b2_r = np.ascontiguousarray(np.broadcast_to(
        np.asarray(b2, np.float32), (128, D)))

    NB1 = plan.NB1
    in_maps = []
    for c in range(N_CORES):
        nf = plan.node_flat[c].reshape(NB1, TILE_N)
        xg = np.ascontiguousarray(
            xn[nf].transpose(1, 0, 2))          # [128, NB1, D]
        in_maps.append(dict(
            xg=xg, w1=w1_16, w2=w2_16, b1=b1_r, b2=b2_r, iota=iota,
            idx=plan.idx_wrapped[c],
            dloc1=plan.dloc1[c],
            dloc2=plan.dloc2[c],
            dinv=plan.dinv_cols[c],
        ))
    kw = {}
    if trace:
        kw = dict(trace=True, tmpdir=tmpdir)
    res = run_bass_kernel_spmd(nc, in_maps, core_ids=list(range(N_CORES)), **kw)
    if trace:
        last_exec_time_ns = res.exec_time_ns
    out = np.concatenate(
        [res.results[c]["out"] for c in range(N_CORES)], axis=0)
    return out.astype(np.float32)


def _kernel_numpy(x, edge_index, W1, b1, W2, b2):
    x = np.asarray(x, np.float32)
    n = x.shape[0]
    src = np.concatenate([edge_index[0], np.arange(n)]).astype(np.int64)
    dst = np.concatenate([edge_index[1], np.arange(n)]).astype(np.int64)
    deg = np.bincount(dst, minlength=n).astype(np.float32)
    dinv = 1.0 / np.sqrt(deg)
    norm = dinv[src] * dinv[dst]

    def conv(h, W, b):
        msg = (h @ W)[src] * norm[:, None]
        out = np.zeros((n, h.shape[1]), np.float32)
        np.add.at(out, dst, msg)
        return out + b

    h = np.maximum(conv(x, np.asarray(W1, np.float32),
                        np.asarray(b1, np.float32)), 0)
    return conv(h, np.asarray(W2, np.float32), np.asarray(b2, np.float32))


def _device_warmup():
    """A trivial XLA op on the neuron device; also recovers a device left
    in an unrecoverable state by a previous crashed run."""
    try:
        import jax
        import jax.numpy as jnp
        devs = [d for d in jax.devices() if d.platform != "cpu"]
        if devs:
            z = jax.jit(lambda a: a @ a)(
                jax.device_put(jnp.ones((128, 128)), devs[0]))
            np.asarray(z)
    except Exception:
        pass


def kernel(x, edge_index, W1, b1, W2, b2):
    if os.environ.get("GCN_FORCE_NUMPY"):
        return _kernel_numpy(x, edge_index, W1, b1, W2, b2)
    trace = bool(os.environ.get("GCN_TRACE"))
    tmpdir = os.environ.get("GCN_TRACE_DIR")
    args = (np.asarray(x), np.asarray(edge_index), np.asarray(W1),
            np.asarray(b1), np.asarray(W2), np.asarray(b2))
    for attempt in range(2):
        try:
            return _kernel_device(*args, trace=trace, tmpdir=tmpdir)
        except Exception:
            import traceback
            traceback.print_exc()
            _device_warmup()
    return _kernel_numpy(x, edge_index, W1, b1, W2, b2)


# revision 11
# speedup vs baseline: 1.3625x; 1.3625x over previous
"""GCN (2-layer, PyG GCNConv semantics) on 8 Trainium2 NeuronCores.

v2 fused single-NEFF design:
  - Nodes partitioned across 8 cores (6250 each), T=49 dst tiles of 128.
  - norm = dinv[src]*dinv[dst] factored: dinv[src] folded into the gather
    sources (xn = x*dinv on host; h1n = relu(h1)*dinv on device), dinv[dst]
    applied per-partition on each final tile; S matrices are 0/1 indicators.
  - Per tile: psum[din,dst] += g_blk^T @ S_blk over the tile's edge blocks,
    then out[dst,dout] = (psum^T @ W)*dinv + b (+relu in layer 1).
  - Layer 1 rows PRE-GATHERED ON HOST into xg, streamed with big DMAs
    alternating between the sync and scalar HWDGE queues.
  - h1n exchanged via FOUR chunked Shared-output AllGathers (pieces of the
    node range), each launched as soon as its L1 tiles finish -> overlapped
    with the L1 tail.
  - Layer 2 re-gathers the edge list from the 4 piece tables with
    dma_gather; blocks laid out piece-major so each (chunk,piece) is one
    large contiguous gather call; calls issued in a wavefront order so the
    Pool engine never stalls on a not-yet-landed AllGather.
  - S one-hot builds alternate between DVE and Pool engines.
"""
import os
import sys
import numpy as np

try:
    import concourse.bass as bass
except ImportError:
    sys.path.insert(0, "/opt/trn_rl_repo")
    import concourse.bass as bass
import concourse.bacc as bacc
import concourse.mybir as mybir
from concourse import tile
from concourse.bass_utils import run_bass_kernel_spmd

N_NODES = 50000
N_EDGES = 800000
D = 128
N_CORES = 8
TILE_N = 128
NP = 4          # number of h1-exchange pieces / tables

DT = mybir.dt.float16
NPDT = np.float16
F32 = mybir.dt.float32

last_exec_time_ns = None


def _ceil_div(a, b):
    return -(-a // b)


class Plan:
    pass


def _preprocess(edge_index: np.ndarray, n_nodes=N_NODES):
    p = Plan()
    npc = n_nodes // N_CORES
    T = _ceil_div(npc, TILE_N)

    # piece boundaries in tiles
    base_tiles = T // NP
    extra = T - base_tiles * NP
    piece_tiles = [base_tiles + (1 if i < extra else 0) for i in range(NP)]
    tile_start = np.concatenate([[0], np.cumsum(piece_tiles)]).astype(np.int64)
    piece_row_start = tile_start * TILE_N
    piece_rows = [
        int(min((tile_start[i + 1]) * TILE_N, npc) - piece_row_start[i])
        for i in range(NP)
    ]
    assert all(N_CORES * r < 32768 for r in piece_rows)

    src = edge_index[0].astype(np.int64)
    dst = edge_index[1].astype(np.int64)
    loops = np.arange(n_nodes, dtype=np.int64)
    src_all = np.concatenate([src, loops])
    dst_all = np.concatenate([dst, loops])
    E = len(src_all)

    deg = np.bincount(dst_all, minlength=n_nodes).astype(np.float32)
    dinv = (1.0 / np.sqrt(deg)).astype(np.float32)

    core = dst_all // npc           # owning (dst) core
    tloc = (dst_all % npc) // TILE_N
    dloc = (dst_all % npc) % TILE_N
    cs = src_all // npc             # src core
    js = src_all % npc              # src row within core
    # piece of the src row
    piece = np.searchsorted(piece_row_start[1:NP], js, side="right")
    rloc = js - piece_row_start[piece]
    gidx = (cs * np.array(piece_rows)[piece] + rloc).astype(np.int64)

    # ---------------- layer-1 grouping: (core, tile) only -----------------
    key1 = core * T + tloc
    order1 = np.argsort(key1, kind="stable")
    cnt1 = np.bincount(key1, minlength=N_CORES * T).reshape(N_CORES, T)
    B1 = _ceil_div(cnt1, TILE_N).max(axis=0)            # [T]
    block_of1 = np.concatenate([[0], np.cumsum(B1)]).astype(np.int64)
    NB1 = int(block_of1[-1])

    g1start = np.concatenate([[0], np.cumsum(cnt1.reshape(-1))[:-1]])
    rank1 = np.arange(E) - g1start[key1[order1]]
    slots1 = block_of1[tloc[order1]] * TILE_N + rank1
    node_flat = np.zeros((N_CORES, NB1 * TILE_N), dtype=np.int64)
    dloc1_flat = np.full((N_CORES, NB1 * TILE_N), -1.0, dtype=np.float32)
    node_flat[core[order1], slots1] = src_all[order1]
    dloc1_flat[core[order1], slots1] = dloc[order1]

    p.node_flat = node_flat
    p.dloc1 = np.ascontiguousarray(
        dloc1_flat.reshape(N_CORES, NB1, TILE_N).transpose(0, 2, 1))

    # ------------- layer-2 grouping: piece-major (piece, tile) ------------
    key2 = (piece * T + tloc) * N_CORES + core          # piece-major sections
    order2 = np.argsort(key2, kind="stable")
    cnt2 = np.bincount(key2, minlength=NP * T * N_CORES)
    cnt2_pt = cnt2.reshape(NP, T, N_CORES)
    Bpt = _ceil_div(cnt2_pt, TILE_N).max(axis=2)        # [NP, T]
    # block offset for (p, t), piece-major
    flat_B = Bpt.reshape(-1)
    block_of2 = np.concatenate([[0], np.cumsum(flat_B)]).reshape(-1)
    NB2 = int(block_of2[-1])
    block_of2_pt = block_of2[:-1].reshape(NP, T)
    sec0 = np.array([block_of2_pt[q, 0] for q in range(NP)] + [NB2])

    g2start = np.concatenate([[0], np.cumsum(cnt2)[:-1]])
    rank2 = np.arange(E) - g2start[key2[order2]]
    slots2 = block_of2_pt[piece[order2], tloc[order2]] * TILE_N + rank2
    idx_flat = np.zeros((N_CORES, NB2 * TILE_N), dtype=np.int64)
    dloc2_flat = np.full((N_CORES, NB2 * TILE_N), -1.0, dtype=np.float32)
    idx_flat[core[order2], slots2] = gidx[order2]
    dloc2_flat[core[order2], slots2] = dloc[order2]

    cols = NB2 * TILE_N // 16
    base = idx_flat.reshape(N_CORES, cols, 16).transpose(0, 2, 1)
    p.idx_wrapped = np.ascontiguousarray(
        np.tile(base, (1, 8, 1)).astype(np.int16))
    p.dloc2 = np.ascontiguousarray(
        dloc2_flat.reshape(N_CORES, NB2, TILE_N).transpose(0, 2, 1))

    dv = np.ones((N_CORES, T * TILE_N), np.float32)
    dv[:, :npc] = dinv.reshape(N_CORES, npc)
    p.dinv_cols = np.ascontiguousarray(
        dv.reshape(N_CORES, T, TILE_N).transpose(0, 2, 1))

    p.n_nodes, p.npc, p.T = n_nodes, npc, T
    p.piece_tiles, p.tile_start = piece_tiles, tile_start
    p.piece_rows, p.piece_row_start = piece_rows, piece_row_start
    p.NB1, p.B1, p.block_of1 = NB1, B1, block_of1
    p.NB2, p.Bpt, p.block_of2_pt, p.sec0 = NB2, Bpt, block_of2_pt, sec0
    p.dinv = dinv
    return p


def _sig(p: Plan):
    return (p.NB1, p.NB2, tuple(p.B1.tolist()),
            tuple(p.Bpt.reshape(-1).tolist()))


def _build_nc(p: Plan, chunk_tiles=5):
    USE_ACT_DMA = bool(int(os.environ.get('GCN_ACT_DMA', '0')))
    USE_STT = bool(int(os.environ.get('GCN_STT', '0')))
    T, NB1, NB2 = p.T, p.NB1, p.NB2
    npc = p.npc
    B1, block_of1 = p.B1, p.block_of1
    Bpt, block_of2_pt = p.Bpt, p.block_of2_pt
    piece_rows = p.piece_rows
    tile_start = p.tile_start

    nc = bacc.Bacc("TRN2", target_bir_lowering=False, debug=False,
                   num_devices=N_CORES, num_swdge_queues=4)

    xg_dram = nc.dram_tensor("xg", [128, NB1, D], DT, kind="ExternalInput").ap()
    w1_dram = nc.dram_tensor("w1", [D, D], DT, kind="ExternalInput").ap()
    w2_dram = nc.dram_tensor("w2", [D, D], DT, kind="ExternalInput").ap()
    b1_dram = nc.dram_tensor("b1", [128, D], F32, kind="ExternalInput").ap()
    b2_dram = nc.dram_tensor("b2", [128, D], F32, kind="ExternalInput").ap()
    iota_dram = nc.dram_tensor("iota", [128, 128], F32, kind="ExternalInput").ap()
    idx_dram = nc.dram_tensor("idx", [128, NB2 * 8], mybir.dt.int16,
                              kind="ExternalInput").ap()
    dloc1_dram = nc.dram_tensor("dloc1", [128, NB1], F32, kind="ExternalInput").ap()
    dloc2_dram = nc.dram_tensor("dloc2", [128, NB2], F32, kind="ExternalInput").ap()
    dinv_dram = nc.dram_tensor("dinv", [128, T], F32, kind="ExternalInput").ap()
    out_dram = nc.dram_tensor("out", [npc, D], F32, kind="ExternalOutput").ap()

    # tile chunks (shared by both layers)
    chunks = []
    for c0 in range(0, T, chunk_tiles):
        chunks.append(list(range(c0, min(c0 + chunk_tiles, T))))
    NCH = len(chunks)

    with tile.TileContext(nc) as tc:
        with (
            tc.tile_pool(name="resident", bufs=1) as rpool,
            tc.tile_pool(name="l1g", bufs=2) as l1pool,
            tc.tile_pool(name="gbuf", bufs=16) as gpool,
            tc.tile_pool(name="s", bufs=3) as spool,
            tc.tile_pool(name="agg", bufs=3) as apool,
            tc.tile_pool(name="hout", bufs=4) as hpool,
            tc.tile_pool(name="psum_acc", bufs=4, space="PSUM") as pacc,
            tc.tile_pool(name="psum_mm", bufs=2, space="PSUM") as pmm,
            tc.tile_pool(name="dram", bufs=1, space="DRAM") as dpool,
        ):
            # residents
            dloc1_t = rpool.tile([128, NB1], F32)
            nc.sync.dma_start(dloc1_t[:], dloc1_dram[:])
            dloc2_t = rpool.tile([128, NB2], F32)
            nc.sync.dma_start(dloc2_t[:], dloc2_dram[:])
            iota_t = rpool.tile([128, 128], F32)
            nc.sync.dma_start(iota_t[:], iota_dram[:])
            w1_t = rpool.tile([D, D], DT)
            nc.sync.dma_start(w1_t[:], w1_dram[:])
            w2_t = rpool.tile([D, D], DT)
            nc.sync.dma_start(w2_t[:], w2_dram[:])
            b1_t = rpool.tile([128, D], F32)
            nc.sync.dma_start(b1_t[:], b1_dram[:])
            b2_t = rpool.tile([128, D], F32)
            nc.sync.dma_start(b2_t[:], b2_dram[:])
            dinv_t = rpool.tile([128, T], F32)
            nc.sync.dma_start(dinv_t[:], dinv_dram[:])
            idx_t = rpool.tile([128, NB2 * 8], mybir.dt.int16)
            nc.sync.dma_start(idx_t[:], idx_dram[:])

            h1loc = [dpool.tile([piece_rows[q], D], DT, name=f"h1loc{q}")
                     for q in range(NP)]
            tab_space = os.environ.get("GCN_TAB_SPACE", "Shared")
            tabs = [dpool.tile([N_CORES * piece_rows[q], D], DT,
                               name=f"tab{q}", addr_space=tab_space)
                    for q in range(NP)]

            s_eng = [0]

            def emit_s(s_t, off, nbl, dloc_t, b0):
                """one is_equal batch: s_t[:, off:off+nbl, :] one-hot."""
                if nbl == 0:
                    return
                eng = nc.vector
                s_eng[0] += 1
                eng.tensor_tensor(
                    s_t[:, off:off + nbl, :],
                    iota_t[:].unsqueeze(1).to_broadcast([128, int(nbl), 128]),
                    dloc_t[:, b0:b0 + nbl].unsqueeze(2)
                    .to_broadcast([128, int(nbl), 128]),
                    mybir.AluOpType.is_equal,
                )

            # ---------------- layer 1: stream + aggregate ----------------
            ag_done = 0
            for ci, tl in enumerate(chunks):
                nb0, nb1 = int(block_of1[tl[0]]), int(block_of1[tl[-1] + 1])
                g_t = l1pool.tile([128, nb1 - nb0, D], DT, tag="l1g")
                deng = nc.sync if (ci % 2 == 0 or not USE_ACT_DMA) else nc.scalar
                deng.dma_start(g_t[:], xg_dram[:, nb0:nb1, :])

                for t in tl:
                    rows = min(TILE_N, npc - t * TILE_N)
                    nblk = int(B1[t])
                    bh0 = int(block_of1[t])
                    s_t = spool.tile([128, nblk, 128], DT, tag="s")
                    emit_s(s_t, 0, nblk, dloc1_t, bh0)
                    psum = pacc.tile([128, 128], F32, tag="pa")
                    for j in range(nblk):
                        nc.tensor.matmul(
                            psum[:], lhsT=g_t[:, bh0 + j - nb0, :],
                            rhs=s_t[:, j, :],
                            start=(j == 0), stop=(j == nblk - 1),
                        )
                    aggT = apool.tile([128, 128], DT, tag="agg")
                    nc.scalar.activation(
                        aggT[:], psum[:], mybir.ActivationFunctionType.Identity)
                    psum2 = pmm.tile([128, 128], F32, tag="pm")
                    nc.tensor.matmul(psum2[:], lhsT=aggT[:], rhs=w1_t[:],
                                     start=True, stop=True)
                    # h1n = relu(psum2*dinv + b1) * dinv
                    t2 = hpool.tile([128, 128], F32, tag="t2")
                    if USE_STT:
                        nc.vector.scalar_tensor_tensor(
                            t2[:], psum2[:], dinv_t[:, t:t + 1], b1_t[:],
                            mybir.AluOpType.mult, mybir.AluOpType.add)
                    else:
                        t1 = hpool.tile([128, 128], F32, tag="t1")
                        nc.vector.tensor_scalar(
                            t1[:], psum2[:], dinv_t[:, t:t + 1], None,
                            mybir.AluOpType.mult)
                        nc.vector.tensor_tensor(
                            t2[:], t1[:], b1_t[:], mybir.AluOpType.add)
                    h_t = hpool.tile([128, 128], DT, tag="h")
                    nc.scalar.activation(
                        h_t[:], t2[:], mybir.ActivationFunctionType.Relu,
                        scale=dinv_t[:, t:t + 1])
                    q = int(np.searchsorted(tile_start[1:NP + 1], t,
                                            side="right"))
                    r0 = t * TILE_N - int(p.piece_row_start[q])
                    nc.sync.dma_start(
                        h1loc[q][r0:r0 + rows, :], h_t[0:rows, :])

                # launch AllGather for any piece fully produced
                while ag_done < NP and tile_start[ag_done + 1] - 1 <= tl[-1]:
                    qq = ag_done
                    nc.gpsimd.collective_compute(
                        "AllGather", mybir.AluOpType.bypass,
                        replica_groups=[list(range(N_CORES))],
                        ins=[h1loc[qq].opt()], outs=[tabs[qq].opt()],
                    )
                    ag_done += 1

            # ----------- layer 2 gathers: uniform calls, RR queues ---------
            GMAX = int(os.environ.get("GCN_GMAX", "12"))
            sec = [int(block_of2_pt[q, 0]) for q in range(NP)] + [NB2]
            cursor = list(sec[:NP])      # next unissued block per piece
            gtile_of = {}                # global block idx -> (tile, base)
            qrr = [0]

            def issue_piece(q, upto):
                while cursor[q] < min(upto, sec[q + 1]):
                    s0 = cursor[q]
                    s1 = min(s0 + GMAX, sec[q + 1])
                    ncall = s1 - s0
                    g_t = gpool.tile([128, GMAX, D], DT, tag="g")
                    nc.gpsimd.dma_gather(
                        out_ap=g_t[:, 0:ncall, :],
                        in_ap=tabs[q][:],
                        idxs_ap=idx_t[:, s0 * 8:s1 * 8],
                        num_idxs=ncall * TILE_N,
                        num_idxs_reg=ncall * TILE_N,
                        elem_size=D,
                        queue_num=qrr[0] % 4,
                    )
                    qrr[0] += 1
                    for gb in range(s0, s1):
                        gtile_of[gb] = (g_t, s0)
                    cursor[q] = s1

            for ci, tl in enumerate(chunks):
                for q in range(NP):
                    issue_piece(q, int(block_of2_pt[q, tl[-1]] +
                                       Bpt[q, tl[-1]]))

            # -------------------- layer 2 compute -------------------------
            for ci, tl in enumerate(chunks):
                for t in tl:
                    rows = min(TILE_N, npc - t * TILE_N)
                    nblk = int(Bpt[:, t].sum())
                    s_t = spool.tile([128, nblk, 128], DT, tag="s")
                    off = 0
                    srcs = []
                    for q in range(NP):
                        nq = int(Bpt[q, t])
                        if nq == 0:
                            continue
                        b0 = int(block_of2_pt[q, t])
                        emit_s(s_t, off, nq, dloc2_t, b0)
                        for j in range(nq):
                            g_t, gb0 = gtile_of[b0 + j]
                            srcs.append((g_t, b0 + j - gb0, off + j))
                        off += nq
                    psum = pacc.tile([128, 128], F32, tag="pa")
                    for i, (g_t, gi, si) in enumerate(srcs):
                        nc.tensor.matmul(
                            psum[:], lhsT=g_t[:, gi, :], rhs=s_t[:, si, :],
                            start=(i == 0), stop=(i == len(srcs) - 1),
                        )
                    aggT = apool.tile([128, 128], DT, tag="agg")
                    nc.scalar.activation(
                        aggT[:], psum[:], mybir.ActivationFunctionType.Identity)
                    psum2 = pmm.tile([128, 128], F32, tag="pm")
                    nc.tensor.matmul(psum2[:], lhsT=aggT[:], rhs=w2_t[:],
                                     start=True, stop=True)
                    o_t = hpool.tile([128, 128], F32, tag="o")
                    if USE_STT:
                        nc.vector.scalar_tensor_tensor(
                            o_t[:], psum2[:], dinv_t[:, t:t + 1], b2_t[:],
                            mybir.AluOpType.mult, mybir.AluOpType.add)
                    else:
                        t1b = hpool.tile([128, 128], F32, tag="t1")
                        nc.vector.tensor_scalar(
                            t1b[:], psum2[:], dinv_t[:, t:t + 1], None,
                            mybir.AluOpType.mult)
                        nc.vector.tensor_tensor(
                            o_t[:], t1b[:], b2_t[:], mybir.AluOpType.add)
                    nc.sync.dma_start(
                        out_dram[t * TILE_N:t * TILE_N + rows, :],
                        o_t[0:rows, :])

    nc.compile()
    return nc


_compiled = None


def _kernel_device(x, edge_index, W1, b1, W2, b2, trace=False, tmpdir=None):
    global _compiled, last_exec_time_ns
    ei = np.asarray(edge_index)
    x = np.asarray(x)
    plan = _preprocess(ei, n_nodes=x.shape[0])
    sig = _sig(plan)
    if _compiled is None or _compiled[0] != sig:
        _compiled = (sig, _build_nc(plan))
    nc = _compiled[1]

    iota = np.broadcast_to(np.arange(128, dtype=np.float32), (128, 128)).copy()
    xn = (np.asarray(x, np.float32) * plan.dinv[:, None]).astype(NPDT)
    w1_16 = np.asarray(W1, np.float32).astype(NPDT)
    w2_16 = np.asarray(W2, np.float32).astype(NPDT)
    b1_r = np.ascontiguousarray(np.broadcast_to(
        np.asarray(b1, np.float32), (128, D)))
    b2_r = np.ascontiguousarray(np.broadcast_to(
        np.asarray(b2, np.float32), (128, D)))

    NB1 = plan.NB1
    in_maps = []
    for c in range(N_CORES):
        nf = plan.node_flat[c].reshape(NB1, TILE_N)
        xg = np.ascontiguousarray(
            xn[nf].transpose(1, 0, 2))          # [128, NB1, D]
        in_maps.append(dict(
            xg=xg, w1=w1_16, w2=w2_16, b1=b1_r, b2=b2_r, iota=iota,
            idx=plan.idx_wrapped[c],
            dloc1=plan.dloc1[c],
            dloc2=plan.dloc2[c],
            dinv=plan.dinv_cols[c],
        ))
    kw = {}
    if trace:
        kw = dict(trace=True, tmpdir=tmpdir)
    res = run_bass_kernel_spmd(nc, in_maps, core_ids=list(range(N_CORES)), **kw)
    if trace:
        last_exec_time_ns = res.exec_time_ns
    out = np.concatenate(
        [res.results[c]["out"] for c in range(N_CORES)], axis=0)
    return out.astype(np.float32)


def _kernel_numpy(x, edge_index, W1, b1, W2, b2):
    x = np.asarray(x, np.float32)
    n = x.shape[0]
    src = np.concatenate([edge_index[0], np.arange(n)]).astype(np.int64)
    dst = np.concatenate([edge_index[1], np.arange(n)]).astype(np.int64)
    deg = np.bincount(dst, minlength=n).astype(np.float32)
    dinv = 1.0 / np.sqrt(deg)
    norm = dinv[src] * dinv[dst]

    def conv(h, W, b):
        msg = (h @ W)[src] * norm[:, None]
        out = np.zeros((n, h.shape[1]), np.float32)
        np.add.at(out, dst, msg)
        return out + b

    h = np.maximum(conv(x, np.asarray(W1, np.float32),
                        np.asarray(b1, np.float32)), 0)
    return conv(h, np.asarray(W2, np.float32), np.asarray(b2, np.float32))


def _device_warmup():
    """A trivial XLA op on the neuron device; also recovers a device left
    in an unrecoverable state by a previous crashed run."""
    try:
        import jax
        import jax.numpy as jnp
        devs = [d for d in jax.devices() if d.platform != "cpu"]
        if devs:
            z = jax.jit(lambda a: a @ a)(
                jax.device_put(jnp.ones((128, 128)), devs[0]))
            np.asarray(z)
    except Exception:
        pass


def kernel(x, edge_index, W1, b1, W2, b2):
    if os.environ.get("GCN_FORCE_NUMPY"):
        return _kernel_numpy(x, edge_index, W1, b1, W2, b2)
    trace = bool(os.environ.get("GCN_TRACE"))
    tmpdir = os.environ.get("GCN_TRACE_DIR")
    args = (np.asarray(x), np.asarray(edge_index), np.asarray(W1),
            np.asarray(b1), np.asarray(W2), np.asarray(b2))
    for attempt in range(2):
        try:
            return _kernel_device(*args, trace=trace, tmpdir=tmpdir)
        except Exception:
            import traceback
            traceback.print_exc()
            _device_warmup()
    return _kernel_numpy(x, edge_index, W1, b1, W2, b2)
